# revision 13
# baseline (speedup 1.0000x reference)
"""Multi-head attention (B=8, N=2048, dim=64, heads=8) on 8 Trainium2 cores.

Sharding: batch-parallel — one batch element per NeuronCore, weights
replicated, no collectives. Per-core flash-style attention, fully
SBUF-resident (no HBM intermediates).
"""
import sys

import numpy as np


def _ensure_path():
    try:
        import concourse  # noqa: F401
    except ImportError:
        for p in (
            "/opt/trn_rl_repo",
            "/root/.axon_site",
            "/root/.axon_site/_ro/trn_rl_repo",
            "/root/.axon_site/_ro/pypackages",
        ):
            if p not in sys.path:
                sys.path.append(p)


_ensure_path()

import concourse.bacc as bacc  # noqa: E402
import concourse.mybir as mybir  # noqa: E402
import concourse.tile as tile  # noqa: E402
from concourse.bass_utils import run_bass_kernel_spmd  # noqa: E402
from concourse.masks import make_identity  # noqa: E402

B, N, D, H = 8, 2048, 64, 8
P = 128
NT = N // P          # 16 n-tiles of 128
IC = N // 512        # 4 query chunks of 512
SCALE = float(D) ** -0.5
F32 = mybir.dt.float32
F32R = mybir.dt.float32r
BF16 = mybir.dt.bfloat16
# j-tile blocks per (head, ichunk): PSUM budget = 2 stile bufs x 3 banks
# + 2 zacc banks = 8 banks.
BLOCKS = [[0, 1, 2], [3, 4, 5], [6, 7, 8], [9, 10, 11], [12, 13, 14], [15]]


def build_program(n_cores=B):
    nc = bacc.Bacc("TRN2", target_bir_lowering=False, debug=False,
                   num_devices=n_cores)
    x_d = nc.dram_tensor("x", [N, D], F32, kind="ExternalInput")
    wqkv_d = nc.dram_tensor("w_qkv", [D, 3 * H * D], F32, kind="ExternalInput")
    wout_d = nc.dram_tensor("w_out", [H * D, D], F32, kind="ExternalInput")
    bout_d = nc.dram_tensor("b_out", [D], F32, kind="ExternalInput")
    out_d = nc.dram_tensor("out", [N, D], F32, kind="ExternalOutput")

    with tile.TileContext(nc) as tc:
        with tc.tile_pool(name="const", bufs=1) as const:
            ident = const.tile([P, P], F32, tag="ident")
            make_identity(nc, ident[:])

            wsb = const.tile([D, 3 * H * D], BF16, tag="wqkv")
            nc.gpsimd.dma_start(wsb[:], wqkv_d.ap())
            wout_sb = const.tile([P, 4, D], BF16, tag="wout")
            nc.gpsimd.dma_start(
                wout_sb[:], wout_d.ap().rearrange("(t p) d -> p t d", p=P))
            b_row = const.tile([1, D], F32, tag="brow")
            nc.sync.dma_start(b_row[:], bout_d.ap().rearrange("(a d) -> a d", a=1))
            b_bc = const.tile([P, D], F32, tag="bbc")
            nc.gpsimd.partition_broadcast(b_bc[:], b_row[:])
            ones3 = const.tile([P, H, 1], F32, tag="ones3")
            nc.gpsimd.memset(ones3[:], 1.0)

            xT = const.tile([D, N], BF16, tag="xT")
            # qk_sb[0..3]: Q^T head-pairs [128, N]; qk_sb[4..7]: K^T pairs
            qk_sb = [const.tile([P, N], BF16, tag=f"qk{i}", name=f"qk{i}")
                     for i in range(8)]
            # V~ per n-tile: [128, H, 65]; col 64 of each head is ones
            vt_sb = [const.tile([P, H, 65], BF16, tag=f"vt{t}", name=f"vt{t}")
                     for t in range(NT)]
            zT = [const.tile([P, N], BF16, tag=f"zT{i}", name=f"zT{i}")
                  for i in range(4)]

            # ---- Phases 1+2 share one PSUM pool so setup tiles release
            # bank-by-bank into the attention pipeline (no phase barrier).
            # Budget: st 3 banks x 2 bufs + za0 + za1 = 8 banks; setup
            # borrows the za0/za1 slots, prefetch borrows st slots.
            with (
                tc.tile_pool(name="xin", bufs=1) as xpool,
                tc.tile_pool(name="spsum", bufs=2,
                             space=bacc.bass.MemorySpace.PSUM) as spsum,
                tc.tile_pool(name="zpsum", bufs=1,
                             space=bacc.bass.MemorySpace.PSUM) as zpsum,
                tc.tile_pool(name="es", bufs=10) as es_pool,
                tc.tile_pool(name="sm", bufs=2) as sm_pool,
            ):
                xall = xpool.tile([P, NT, D], F32, tag="xall")
                nc.sync.dma_start(
                    xall[:], x_d.ap().rearrange("(t p) d -> p t d", p=P))

                alt = [0]

                def setup_psum():
                    # borrow the za0/za1 single-bank slots for setup matmuls
                    alt[0] ^= 1
                    return zpsum.tile([P, 512], F32, tag=f"za{alt[0]}",
                                      name="mps")

                def emit_qk(ct, icxs, pool_fn):
                    w_sl = wsb[:, ct * P:(ct + 1) * P]
                    for icx in icxs:
                        mp = pool_fn()
                        nc.tensor.matmul(
                            mp[0:P, 0:512], w_sl,
                            xT[:, icx * 512:(icx + 1) * 512],
                            start=True, stop=True)
                        nc.vector.tensor_copy(
                            qk_sb[ct][:, icx * 512:(icx + 1) * 512],
                            mp[0:P, 0:512])

                for g in range(IC):
                    for t in range(4 * g, 4 * g + 4):
                        pp = setup_psum()
                        nc.tensor.transpose(pp[0:D, 0:P], xall[:, t, :],
                                            ident[:])
                        nc.vector.tensor_copy(xT[:, t * P:(t + 1) * P],
                                              pp[0:D, 0:P])
                    emit_qk(4, [g], setup_psum)
                    emit_qk(0, [g], setup_psum)
                for t in range(NT):
                    mp = setup_psum()
                    nc.tensor.matmul(
                        mp[0:P, 0:512], xT[:, t * P:(t + 1) * P],
                        wsb[:, 2 * H * D:3 * H * D],
                        start=True, stop=True)
                    nc.vector.tensor_copy(vt_sb[t][:, :, 64:65], ones3[:])
                    nc.vector.tensor_copy(
                        vt_sb[t][:, :, 0:64],
                        mp[0:P, 0:512].rearrange("p (h d) -> p h d", h=H))
                # Head-pair interleaving: heads 2k / 2k+1 live in
                # complementary partition halves, so their K=64 S-matmuls
                # use disjoint PE row-groups and execute concurrently.
                # Software pipeline: the A@V matmuls of chunk k are emitted
                # after the exp of chunk k+1 so PE never waits on ACT.
                pending = None  # (es_tile, chunk, za, hp)

                def flush_av(nc):
                    nonlocal pending
                    if pending is None:
                        return
                    es_p, chunk_p, za_p, hp_p = pending
                    for ci, (hh, j) in enumerate(chunk_p):
                        nc.tensor.matmul(
                            za_p[hh][:], vt_sb[j][:, 2 * hp_p + hh, :],
                            es_p[:, ci * 512:(ci + 1) * 512],
                            start=(jc[hh] == 0), stop=(jc[hh] == NT - 1),
                            skip_group_check=True)
                        jc[hh] += 1
                    pending = None

                # interleaved (half, j) slices chunked by 3 (= 3 PSUM banks)
                slices = [(hh, j) for j in range(NT) for hh in (0, 1)]
                chunks = [slices[i:i + 3] for i in range(0, 2 * NT, 3)]

                def st_psum():
                    return spsum.tile([P, 512], F32, tag="st", name="mpa")

                for hp in range(H // 2):
                    qt = qk_sb[hp]
                    kt = qk_sb[4 + hp]
                    for icx in range(IC):
                        # prefetch next head-pair's Q/K tiles, two matmuls
                        # per icx, borrowing an st slot briefly
                        if hp + 1 < H // 2:
                            emit_qk(4 + hp + 1, [icx], st_psum)
                            emit_qk(hp + 1, [icx], st_psum)
                        za = [zpsum.tile([65, 512], F32, tag="za0",
                                         name="za0", bufs=1),
                              zpsum.tile([65, 512], F32, tag="za1",
                                         name="za1", bufs=1)]
                        jc = [0, 0]
                        for chunk in chunks:
                            st = spsum.tile([P, 512 * len(chunk)], F32,
                                            tag="st", name="st")
                            es = es_pool.tile([P, 512 * len(chunk)], BF16,
                                              tag="es", name="es")
                            for ci, (hh, j) in enumerate(chunk):
                                r0 = hh * 64
                                nc.tensor.matmul(
                                    st[:, ci * 512:(ci + 1) * 512],
                                    kt[r0:r0 + 64, j * P:(j + 1) * P],
                                    qt[r0:r0 + 64,
                                       icx * 512:(icx + 1) * 512],
                                    start=True, stop=True)
                            nc.scalar.activation(
                                es[:], st[:],
                                mybir.ActivationFunctionType.Exp, scale=SCALE)
                            flush_av(nc)
                            pending = (es, chunk, za, hp)
                        flush_av(nc)
                        # stage both z's out of PSUM first so the za banks
                        # free immediately; then the slow recip chain
                        zus = []
                        for hh in (0, 1):
                            zu = sm_pool.tile([65, 512], F32, tag=f"zu{hh}",
                                              name=f"zu{hh}")
                            nc.vector.tensor_copy(zu[:], za[hh][:])
                            zus.append(zu)
                        for hh in (0, 1):
                            rc = sm_pool.tile([1, 512], F32, tag="rc",
                                              name="rc")
                            nc.vector.reciprocal(rc[:], zus[hh][64:65, :])
                            bc = sm_pool.tile([64, 512], F32, tag="bc",
                                              name="bc")
                            nc.gpsimd.partition_broadcast(bc[:], rc[:])
                            nc.vector.tensor_mul(
                                zT[hp][hh * 64:hh * 64 + 64,
                                       icx * 512:(icx + 1) * 512],
                                zus[hh][0:64, :], bc[:])

            # ---- Phase 3: output projection ----
            with (
                tc.tile_pool(name="opsum", bufs=2,
                             space=bacc.bass.MemorySpace.PSUM) as opsum,
                tc.tile_pool(name="outp", bufs=3) as outp,
            ):
                for t in range(NT):
                    op = opsum.tile([P, D], F32, tag="op")
                    for ct in range(4):
                        nc.tensor.matmul(
                            op[:], zT[ct][:, t * P:(t + 1) * P],
                            wout_sb[:, ct, :],
                            start=(ct == 0), stop=(ct == 3),
                            skip_group_check=True)
                    ot = outp.tile([P, D], F32, tag="ot")
                    nc.vector.tensor_add(ot[:], op[:], b_bc[:])
                    nc.sync.dma_start(out_d.ap()[t * P:(t + 1) * P, :], ot[:])

    nc.compile()
    return nc


_PROG = None


def _get_program():
    global _PROG
    if _PROG is None:
        _PROG = build_program()
    return _PROG


def kernel(x, W_qkv, W_out, b_out):
    nc = _get_program()
    x = np.asarray(x, dtype=np.float32)
    wq = np.ascontiguousarray(np.asarray(W_qkv, dtype=np.float32))
    wo = np.ascontiguousarray(np.asarray(W_out, dtype=np.float32))
    bo = np.ascontiguousarray(np.asarray(b_out, dtype=np.float32))
    in_maps = [
        {"x": np.ascontiguousarray(x[i]), "w_qkv": wq, "w_out": wo,
         "b_out": bo}
        for i in range(B)
    ]
    res = run_bass_kernel_spmd(nc, in_maps, list(range(B)))
    return np.stack([res.results[i]["out"] for i in range(B)], axis=0)


# revision 15
# speedup vs baseline: 1.2209x; 1.2209x over previous
"""Multi-head attention (B=8, N=2048, dim=64, heads=8) on 8 Trainium2 cores.

Sharding: batch-parallel — one batch element per NeuronCore, weights
replicated, no collectives. Per-core flash-style attention, fully
SBUF-resident (no HBM intermediates).
"""
import sys

import numpy as np


def _ensure_path():
    try:
        import concourse  # noqa: F401
    except ImportError:
        for p in (
            "/opt/trn_rl_repo",
            "/root/.axon_site",
            "/root/.axon_site/_ro/trn_rl_repo",
            "/root/.axon_site/_ro/pypackages",
        ):
            if p not in sys.path:
                sys.path.append(p)


_ensure_path()

import concourse.bacc as bacc  # noqa: E402
import concourse.mybir as mybir  # noqa: E402
import concourse.tile as tile  # noqa: E402
from concourse.bass_utils import run_bass_kernel_spmd  # noqa: E402
from concourse.masks import make_identity  # noqa: E402

B, N, D, H = 8, 2048, 64, 8
P = 128
NT = N // P          # 16 n-tiles of 128
IC = N // 512        # 4 query chunks of 512
SCALE = float(D) ** -0.5
F32 = mybir.dt.float32
F32R = mybir.dt.float32r
BF16 = mybir.dt.bfloat16
# j-tile blocks per (head, ichunk): PSUM budget = 2 stile bufs x 3 banks
# + 2 zacc banks = 8 banks.
BLOCKS = [[0, 1, 2], [3, 4, 5], [6, 7, 8], [9, 10, 11], [12, 13, 14], [15]]


def build_program(n_cores=B):
    nc = bacc.Bacc("TRN2", target_bir_lowering=False, debug=False,
                   num_devices=n_cores)
    x_d = nc.dram_tensor("x", [N, D], F32, kind="ExternalInput")
    wqkv_d = nc.dram_tensor("w_qkv", [D, 3 * H * D], F32, kind="ExternalInput")
    wout_d = nc.dram_tensor("w_out", [H * D, D], F32, kind="ExternalInput")
    bout_d = nc.dram_tensor("b_out", [D], F32, kind="ExternalInput")
    out_d = nc.dram_tensor("out", [N, D], F32, kind="ExternalOutput")

    with tile.TileContext(nc) as tc:
        with tc.tile_pool(name="const", bufs=1) as const:
            ident = const.tile([P, P], F32, tag="ident")
            make_identity(nc, ident[:])

            wsb = const.tile([D, 3 * H * D], BF16, tag="wqkv")
            nc.gpsimd.dma_start(wsb[:], wqkv_d.ap())
            wout_sb = const.tile([P, 4, D], BF16, tag="wout")
            nc.gpsimd.dma_start(
                wout_sb[:], wout_d.ap().rearrange("(t p) d -> p t d", p=P))
            b_row = const.tile([1, D], F32, tag="brow")
            nc.sync.dma_start(b_row[:], bout_d.ap().rearrange("(a d) -> a d", a=1))
            b_bc = const.tile([P, D], F32, tag="bbc")
            nc.gpsimd.partition_broadcast(b_bc[:], b_row[:])
            ones3 = const.tile([P, H, 1], F32, tag="ones3")
            nc.gpsimd.memset(ones3[:], 1.0)

            xT = const.tile([D, N], BF16, tag="xT")
            # qk_sb[0..3]: Q^T head-pairs [128, N]; qk_sb[4..7]: K^T pairs
            qk_sb = [const.tile([P, N], BF16, tag=f"qk{i}", name=f"qk{i}")
                     for i in range(8)]
            # V~ per n-tile: [128, H, 65]; col 64 of each head is ones
            vt_sb = [const.tile([P, H, 65], BF16, tag=f"vt{t}", name=f"vt{t}")
                     for t in range(NT)]
            zT = [const.tile([P, N], BF16, tag=f"zT{i}", name=f"zT{i}")
                  for i in range(4)]

            # ---- Phases 1+2 share one PSUM pool so setup tiles release
            # bank-by-bank into the attention pipeline (no phase barrier).
            # Budget: st 3 banks x 2 bufs + za0 + za1 = 8 banks; setup
            # borrows the za0/za1 slots, prefetch borrows st slots.
            with (
                tc.tile_pool(name="xin", bufs=1) as xpool,
                tc.tile_pool(name="spsum", bufs=2,
                             space=bacc.bass.MemorySpace.PSUM) as spsum,
                tc.tile_pool(name="zpsum", bufs=1,
                             space=bacc.bass.MemorySpace.PSUM) as zpsum,
                tc.tile_pool(name="es", bufs=10) as es_pool,
                tc.tile_pool(name="sm", bufs=2) as sm_pool,
            ):
                xall = xpool.tile([P, NT, D], F32, tag="xall")
                nc.sync.dma_start(
                    xall[:], x_d.ap().rearrange("(t p) d -> p t d", p=P))

                alt = [0]

                def setup_psum():
                    # borrow the za0/za1 single-bank slots for setup matmuls
                    alt[0] ^= 1
                    return zpsum.tile([P, 512], F32, tag=f"za{alt[0]}",
                                      name="mps")

                def emit_qk(ct, icxs, pool_fn):
                    w_sl = wsb[:, ct * P:(ct + 1) * P]
                    for icx in icxs:
                        mp = pool_fn()
                        nc.tensor.matmul(
                            mp[0:P, 0:512], w_sl,
                            xT[:, icx * 512:(icx + 1) * 512],
                            start=True, stop=True)
                        nc.vector.tensor_copy(
                            qk_sb[ct][:, icx * 512:(icx + 1) * 512],
                            mp[0:P, 0:512])

                for g in range(IC):
                    for t in range(4 * g, 4 * g + 4):
                        pp = setup_psum()
                        nc.tensor.transpose(pp[0:D, 0:P], xall[:, t, :],
                                            ident[:])
                        nc.vector.tensor_copy(xT[:, t * P:(t + 1) * P],
                                              pp[0:D, 0:P])
                    emit_qk(4, [g], setup_psum)
                    emit_qk(0, [g], setup_psum)
                for t in range(NT):
                    mp = setup_psum()
                    nc.tensor.matmul(
                        mp[0:P, 0:512], xT[:, t * P:(t + 1) * P],
                        wsb[:, 2 * H * D:3 * H * D],
                        start=True, stop=True)
                    nc.vector.tensor_copy(vt_sb[t][:, :, 64:65], ones3[:])
                    nc.vector.tensor_copy(
                        vt_sb[t][:, :, 0:64],
                        mp[0:P, 0:512].rearrange("p (h d) -> p h d", h=H))
                # Head-pair interleaving: heads 2k / 2k+1 live in
                # complementary partition halves, so their K=64 S-matmuls
                # use disjoint PE row-groups and execute concurrently.
                # Software pipeline: the A@V matmuls of chunk k are emitted
                # after the exp of chunk k+1 so PE never waits on ACT.
                pending = None  # (es_tile, chunk, za, hp)

                def flush_av(nc):
                    nonlocal pending
                    if pending is None:
                        return
                    es_p, chunk_p, za_p, hp_p = pending
                    for ci, (hh, j) in enumerate(chunk_p):
                        nc.tensor.matmul(
                            za_p[hh][:], vt_sb[j][:, 2 * hp_p + hh, :],
                            es_p[:, ci * 512:(ci + 1) * 512],
                            start=(jc[hh] == 0), stop=(jc[hh] == NT - 1),
                            skip_group_check=True)
                        jc[hh] += 1
                    pending = None

                # interleaved (half, j) slices chunked by 3 (= 3 PSUM banks)
                slices = [(hh, j) for j in range(NT) for hh in (0, 1)]
                chunks = [slices[i:i + 3] for i in range(0, 2 * NT, 3)]

                # Normalization is emitted one iteration late, sandwiched
                # between the next iteration's prefetch copy and its zu
                # copies, so the slow DVE reciprocal chain never blocks an
                # op that is holding a PSUM slot.
                norm_pending = None  # (zus, hp, icx)

                def flush_norm(nc):
                    nonlocal norm_pending
                    if norm_pending is None:
                        return
                    zus_p, hp_p, icx_p = norm_pending
                    for hh in (0, 1):
                        rc = sm_pool.tile([1, 512], F32, tag="rc",
                                          name="rc")
                        nc.vector.reciprocal(rc[:], zus_p[hh][64:65, :])
                        bc = sm_pool.tile([64, 512], F32, tag="bc",
                                          name="bc")
                        nc.gpsimd.partition_broadcast(bc[:], rc[:])
                        nc.vector.tensor_mul(
                            zT[hp_p][hh * 64:hh * 64 + 64,
                                     icx_p * 512:(icx_p + 1) * 512],
                            zus_p[hh][0:64, :], bc[:])
                    norm_pending = None

                for hp in range(H // 2):
                    qt = qk_sb[hp]
                    kt = qk_sb[4 + hp]
                    for icx in range(IC):
                        # prefetch next head-pair's Q/K tiles, two matmuls
                        # per icx, borrowing the za slots briefly
                        if hp + 1 < H // 2:
                            emit_qk(4 + hp + 1, [icx], setup_psum)
                            emit_qk(hp + 1, [icx], setup_psum)
                        flush_norm(nc)
                        za = [zpsum.tile([65, 512], F32, tag="za0",
                                         name="za0", bufs=1),
                              zpsum.tile([65, 512], F32, tag="za1",
                                         name="za1", bufs=1)]
                        jc = [0, 0]
                        for chunk in chunks:
                            st = spsum.tile([P, 512 * len(chunk)], F32,
                                            tag="st", name="st")
                            es = es_pool.tile([P, 512 * len(chunk)], BF16,
                                              tag="es", name="es")
                            for ci, (hh, j) in enumerate(chunk):
                                r0 = hh * 64
                                nc.tensor.matmul(
                                    st[:, ci * 512:(ci + 1) * 512],
                                    kt[r0:r0 + 64, j * P:(j + 1) * P],
                                    qt[r0:r0 + 64,
                                       icx * 512:(icx + 1) * 512],
                                    start=True, stop=True)
                            nc.scalar.activation(
                                es[:], st[:],
                                mybir.ActivationFunctionType.Exp, scale=SCALE)
                            flush_av(nc)
                            pending = (es, chunk, za, hp)
                        flush_av(nc)
                        # stage both z's out of PSUM so the za banks free
                        # immediately; normalization deferred one iteration
                        zus = []
                        for hh in (0, 1):
                            zu = sm_pool.tile([65, 512], F32, tag=f"zu{hh}",
                                              name=f"zu{hh}")
                            nc.vector.tensor_copy(zu[:], za[hh][:])
                            zus.append(zu)
                        norm_pending = (zus, hp, icx)
                flush_norm(nc)

            # ---- Phase 3: output projection ----
            with (
                tc.tile_pool(name="opsum", bufs=2,
                             space=bacc.bass.MemorySpace.PSUM) as opsum,
                tc.tile_pool(name="outp", bufs=3) as outp,
            ):
                for t in range(NT):
                    op = opsum.tile([P, D], F32, tag="op")
                    for ct in range(4):
                        nc.tensor.matmul(
                            op[:], zT[ct][:, t * P:(t + 1) * P],
                            wout_sb[:, ct, :],
                            start=(ct == 0), stop=(ct == 3),
                            skip_group_check=True)
                    ot = outp.tile([P, D], F32, tag="ot")
                    nc.vector.tensor_add(ot[:], op[:], b_bc[:])
                    nc.sync.dma_start(out_d.ap()[t * P:(t + 1) * P, :], ot[:])

    nc.compile()
    return nc


_PROG = None


def _get_program():
    global _PROG
    if _PROG is None:
        _PROG = build_program()
    return _PROG


def kernel(x, W_qkv, W_out, b_out):
    nc = _get_program()
    x = np.asarray(x, dtype=np.float32)
    wq = np.ascontiguousarray(np.asarray(W_qkv, dtype=np.float32))
    wo = np.ascontiguousarray(np.asarray(W_out, dtype=np.float32))
    bo = np.ascontiguousarray(np.asarray(b_out, dtype=np.float32))
    in_maps = [
        {"x": np.ascontiguousarray(x[i]), "w_qkv": wq, "w_out": wo,
         "b_out": bo}
        for i in range(B)
    ]
    res = run_bass_kernel_spmd(nc, in_maps, list(range(B)))
    return np.stack([res.results[i]["out"] for i in range(B)], axis=0)
